# revision 17
# baseline (speedup 1.0000x reference)
"""TRN2 Bass kernel for nn_BSLinear_71159018160311.

Computes  out = input @ W.T  with
  W = U @ diag(weight^2 * mask) @ Vh + U_additional @ Vh_additional

Sharding: data-parallel over the B*S=16384 token dim across 8 NeuronCores
(2048 tokens/core), no collectives. Each core runs the factorized form as
two fused matmul phases:

  phase 1 (bf16 in, fp32 accumulate):
           t = V_eff @ x_c.T   kept entirely in SBUF (r-major, [RP, 2048])
  phase 2 (fp32r, full-rate fp32 streaming on the PE):
           yT_c = U_eff @ t    (ut streamed once; output dout-major, host
           transposes back)

Schedule: ONE 8-bank PSUM pool for the whole kernel; every accumulation
group is [128, 512] (one bank, one 512-token chunk), drained right after
it completes. Phase 1 runs token-chunk-outer / r-inner per k-block (the 8
r-accumulators occupy all 8 banks per pass); phase 2 is a flat stream of
(dout-128-row, token-chunk) groups. Consequences:
  - bank reuse distance is always >= 8 groups, so no matmul ever waits on
    a drain (PSUM WAR deps are tile-granular in the Tile framework)
  - no PSUM pool boundaries -> the phase-1 -> phase-2 transition is free
  - the tail drain chain after the very last matmul is one 512-chunk copy
    + one 256KB store
Cold start: warm-up matmuls (reading an SBUF tile that is only written
later, so they have no dependencies) hold the PE p-state ramp while the
first DMAs land; the first k-block's DMAs are issued in exact consumption
order with the leading tiles split small. Inputs x/vt are bf16 (halves
phase-1 HBM traffic and SBUF footprint; PSUM accumulation and everything
downstream stays fp32, adding only ~3e-3 relative error).

V_eff = [Vh; Vh_additional(pad)]  (rows), U_eff = [U*s, U_additional(pad)]
(cols), s = weight^2*mask folded on host. When U_additional/Vh_additional
are all-zero (they are for this problem instance), the padded tail is
dropped (NR=8 -> RP=1024), saving 11% of the matmul work; otherwise the
NR=9 (RP=1152) program handles the full module.
"""

import functools

import numpy as np

B, S, D_IN, D_OUT, R, A = 4, 4096, 4096, 4096, 1024, 64
N_CORES = 8
T = B * S
TC = T // N_CORES  # 2048
KT = D_IN // 128  # 32
KB = 4
NB = KT // KB
NN = TC // 512  # 4
ND = D_OUT // 512  # 8
N_WARM = 7  # warm-up matmuls covering the DMA cold start


@functools.lru_cache(maxsize=2)
def _build(NR):
    import concourse.bacc as bacc
    import concourse.mybir as mybir
    import concourse.tile as tile

    RP = NR * 128
    f32r = mybir.dt.float32r
    f32 = mybir.dt.float32
    bf16 = mybir.dt.bfloat16
    add = mybir.AluOpType.add

    nc = bacc.Bacc(trn_type="TRN2")
    with tile.TileContext(nc) as tc:
        with tc.tile_pool(name="dram", bufs=1, space="DRAM") as dram:
            xT = dram.tile([D_IN, TC], bf16, kind="ExternalInput", name="xT")
            vt = dram.tile([D_IN, RP], bf16, kind="ExternalInput", name="vt")
            ut = dram.tile([RP, D_OUT], f32r, kind="ExternalInput", name="ut")
            yT = dram.tile([D_OUT, TC], f32, kind="ExternalOutput", name="yT")

            with (
                tc.tile_pool(name="tsb", bufs=NR) as tpool,
                tc.tile_pool(name="ut0", bufs=1) as u0pool,
            ):
                t_sb = [tpool.tile([128, TC], f32r, name="tsb") for _ in range(NR)]
                # first ut chunk: loads during phase 1 (own address space)
                ut0 = u0pool.tile([128, NR, 512], f32r)

                # single PSUM pool for the whole kernel: 8 x 1-bank tiles
                pspool = tc.alloc_tile_pool(name="ps", bufs=8, space="PSUM")

                # warm-up: no deps (t_sb[0] is only written later); fills the
                # PE pipe + p-state ramp while the first DMAs land
                for _ in range(N_WARM):
                    wp = pspool.tile([128, 512], f32, name="ps")
                    nc.tensor.matmul(
                        wp[:],
                        lhsT=t_sb[0][:, 0:128],
                        rhs=t_sb[0][:, 0:512],
                        start=True,
                        stop=True,
                    )

                # ---- phase 1 ----
                xpool = tc.alloc_tile_pool(name="xk", bufs=2 * KB)
                vpool = tc.alloc_tile_pool(name="vk", bufs=2 * KB)

                # blocks 0/1: tiles pre-allocated, DMAs issued in consumption
                # order (block 0's x in 512-col chunks, leading vt piece small)
                xts01 = [xpool.tile([128, TC], bf16, name="xk") for _ in range(2 * KB)]
                vts01 = [vpool.tile([128, RP], bf16, name="vk") for _ in range(2 * KB)]
                nc.sync.dma_start(xts01[0][:, 0:512], xT[0:128, 0:512])
                nc.sync.dma_start(vts01[0][:, 0:128], vt[0:128, 0:128])
                nc.sync.dma_start(vts01[0][:, 128:RP], vt[0:128, 128:RP])
                for j in range(1, KB):
                    nc.sync.dma_start(vts01[j][:], vt[j * 128:(j + 1) * 128, :])
                    nc.sync.dma_start(xts01[j][:, 0:512], xT[j * 128:(j + 1) * 128, 0:512])
                for n in range(1, NN):
                    for j in range(KB):
                        nc.sync.dma_start(
                            xts01[j][:, n * 512:(n + 1) * 512],
                            xT[j * 128:(j + 1) * 128, n * 512:(n + 1) * 512],
                        )
                    # one block-1 (x, vt) whole-tile pair between chunk quads
                    jb = KB + n - 1
                    nc.sync.dma_start(xts01[jb][:], xT[jb * 128:(jb + 1) * 128, :])
                    nc.sync.dma_start(vts01[jb][:], vt[jb * 128:(jb + 1) * 128, :])
                jb = 2 * KB - 1
                nc.sync.dma_start(xts01[jb][:], xT[jb * 128:(jb + 1) * 128, :])
                nc.sync.dma_start(vts01[jb][:], vt[jb * 128:(jb + 1) * 128, :])
                nc.sync.dma_start(
                    ut0[:], ut[:, 0:512].rearrange("(ko p) f -> p ko f", p=128)
                )

                # uniform n-outer / r-inner blocks, one bank per (n, r) group
                for kb in range(NB):
                    if kb < 2:
                        xts = xts01[kb * KB:(kb + 1) * KB]
                        vts = vts01[kb * KB:(kb + 1) * KB]
                    else:
                        xts, vts = [], []
                        for j in range(KB):
                            k = kb * KB + j
                            xt_t = xpool.tile([128, TC], bf16, name="xk")
                            nc.sync.dma_start(xt_t[:], xT[k * 128:(k + 1) * 128, :])
                            vt_t = vpool.tile([128, RP], bf16, name="vk")
                            nc.sync.dma_start(vt_t[:], vt[k * 128:(k + 1) * 128, :])
                            xts.append(xt_t)
                            vts.append(vt_t)
                    for n in range(NN):
                        ps_n = [
                            pspool.tile([128, 512], f32, name="ps") for _ in range(NR)
                        ]
                        for j in range(KB):
                            for r in range(NR):
                                nc.tensor.matmul(
                                    ps_n[r][:],
                                    lhsT=vts[j][:, r * 128:(r + 1) * 128],
                                    rhs=xts[j][:, n * 512:(n + 1) * 512],
                                    start=(j == 0),
                                    stop=(j == KB - 1),
                                )
                        sl = slice(n * 512, (n + 1) * 512)
                        if kb == 0:
                            for r in range(NR):
                                nc.any.tensor_copy(t_sb[r][:, sl], ps_n[r][:])
                        else:
                            for r in range(NR):
                                nc.any.tensor_tensor(
                                    t_sb[r][:, sl], t_sb[r][:, sl], ps_n[r][:], add
                                )
                vpool.release()
                xpool.release()

                # ---- phase 2 (ut stationary, t moving; dout-major out) ----
                with (
                    tc.tile_pool(name="utd", bufs=2) as upool,
                    tc.tile_pool(name="ysb", bufs=8) as ypool,
                ):
                    for d in range(ND):
                        if d == 0:
                            ut_t = ut0
                        else:
                            ut_t = upool.tile([128, NR, 512], f32r, name="utd")
                            nc.sync.dma_start(
                                ut_t[:],
                                ut[:, d * 512:(d + 1) * 512].rearrange(
                                    "(ko p) f -> p ko f", p=128
                                ),
                            )
                        for dd in range(4):  # 128-wide dout sub-blocks
                            row = d * 512 + dd * 128
                            for n in range(NN):
                                final = d == ND - 1 and dd == 3 and n == NN - 1
                                # very last chunk: two 256-token sub-groups so
                                # the post-last-matmul drain chain is half as
                                # long (256 keeps fp32r at full rate)
                                subs = ((0, 256), (256, 512)) if final else ((0, 512),)
                                for lo, hi in subs:
                                    pt = pspool.tile([128, 512], f32, name="ps")
                                    for r in range(NR):
                                        nc.tensor.matmul(
                                            pt[:, lo:hi],
                                            lhsT=ut_t[:, r, dd * 128:(dd + 1) * 128],
                                            rhs=t_sb[r][:, n * 512 + lo:n * 512 + hi],
                                            start=(r == 0),
                                            stop=(r == NR - 1),
                                        )
                                    ysb = ypool.tile([128, hi - lo], f32, name="ysb")
                                    nc.any.tensor_copy(ysb[:], pt[:, lo:hi])
                                    nc.sync.dma_start(
                                        yT[row : row + 128,
                                           n * 512 + lo:n * 512 + hi],
                                        ysb[:],
                                    )
                pspool.release()
    nc.compile()
    return nc, xT.name, vt.name, ut.name, yT.name


def _prep_maps(input, weight, U, Vh, U_additional, Vh_additional, mask, names, NR):
    xT_name, vt_name, ut_name = names
    RP = NR * 128
    s = weight * weight * mask
    U_eff = np.zeros((D_OUT, RP), np.float32)
    U_eff[:, :R] = U * s[None, :]
    V_eff = np.zeros((RP, D_IN), np.float32)
    V_eff[:R] = Vh
    if NR > R // 128:
        U_eff[:, R : R + A] = U_additional
        V_eff[R : R + A] = Vh_additional
    import ml_dtypes

    bf16 = ml_dtypes.bfloat16
    # phase 1 runs in bf16 (x, vt quantized here; accumulation stays fp32 in
    # PSUM / t_sb): halves phase-1 HBM traffic, adds ~0.3% relative error
    vt = np.ascontiguousarray(V_eff.T).astype(bf16)
    ut = np.ascontiguousarray(U_eff.T)
    x2 = np.asarray(input, dtype=np.float32).reshape(T, D_IN)
    in_maps = []
    for c in range(N_CORES):
        xTc = np.ascontiguousarray(x2[c * TC : (c + 1) * TC].T).astype(bf16)
        in_maps.append({xT_name: xTc, vt_name: vt, ut_name: ut})
    return in_maps


def _gather(results, yT_name):
    out = np.empty((T, D_OUT), np.float32)
    for c in range(N_CORES):
        out[c * TC : (c + 1) * TC] = results[c][yT_name].T
    return out.reshape(B, S, D_OUT)


def _pick_nr(U_additional, Vh_additional):
    if not np.asarray(U_additional).any() or not np.asarray(Vh_additional).any():
        return R // 128  # additional term contributes nothing
    return (R + A + 127) // 128


def kernel(input, weight, U, Vh, U_additional, Vh_additional, mask, **_kw):
    from concourse.bass_utils import run_bass_kernel_spmd

    input = np.asarray(input, dtype=np.float32)
    weight = np.asarray(weight, dtype=np.float32)
    U = np.asarray(U, dtype=np.float32)
    Vh = np.asarray(Vh, dtype=np.float32)
    U_additional = np.asarray(U_additional, dtype=np.float32)
    Vh_additional = np.asarray(Vh_additional, dtype=np.float32)
    mask = np.asarray(mask, dtype=np.float32)

    NR = _pick_nr(U_additional, Vh_additional)
    nc, xT_name, vt_name, ut_name, yT_name = _build(NR)
    in_maps = _prep_maps(
        input, weight, U, Vh, U_additional, Vh_additional, mask,
        (xT_name, vt_name, ut_name), NR,
    )
    res = run_bass_kernel_spmd(nc, in_maps, core_ids=list(range(N_CORES)))
    return _gather(res.results, yT_name)


# revision 19
# speedup vs baseline: 1.1248x; 1.1248x over previous
"""TRN2 Bass kernel for nn_BSLinear_71159018160311 — fp8 DoubleRow phase 1.

out = input @ W.T,  W = U @ diag(weight^2 * mask) @ Vh (+ additional term).

Data-parallel over B*S=16384 tokens across 8 cores (2048 tokens/core), no
collectives. Factorized two-phase form per core:

  phase 1: t = V_eff @ x_c.T  in fp8e4m3 DoubleRow (256-deep contraction,
           0.5 PE cycles/row) with hi/lo splitting of BOTH operands:
           A@B ~= A1B1 + A1B2 + A2B1 (lo*lo dropped), all three products
           accumulated in the same PSUM group -> 0.75 cycles/row at ~8
           significand bits per operand, vs 1.0 for bf16/fp32r.
           vt is pre-scaled by 64 on host (fp8 dynamic range), the inverse
           is folded into ut. PSUM/t_sb stay fp32.
  phase 2: yT_c = U_eff @ t  in fp32r (full-rate fp32), t never leaves SBUF.

Schedule: ONE 8-bank PSUM pool; every accumulation group is [128, 512]
(one bank, one 512-token chunk), drained right after it completes
(bank reuse distance >= 8 groups -> no matmul ever WAR-waits on a drain;
tile-granular WAR deps). Warm-up matmuls (no deps) hold the PE p-state
ramp while the first DMAs land; first-block DMAs are issued in exact
consumption order with the leading pieces split small. The very last
output chunk is drained as two 256-token sub-groups to halve the tail
chain.
"""

import functools

import numpy as np

B, S, D_IN, D_OUT, R, A = 4, 4096, 4096, 4096, 1024, 64
N_CORES = 8
T = B * S
TC = T // N_CORES  # 2048
KP = D_IN // 256  # 16 k-pairs (256-deep DR contraction each)
KPB = 2  # k-pairs per phase-1 block
NBP = KP // KPB  # 8 blocks
NN = TC // 512  # 4
ND = D_OUT // 512  # 8
N_WARM = 8  # warm-up matmuls covering the DMA cold start
VSCALE = 64.0  # host pre-scale of vt into fp8 range; inverse folded into ut


@functools.lru_cache(maxsize=2)
def _build(NR):
    import concourse.bacc as bacc
    import concourse.mybir as mybir
    import concourse.tile as tile

    RP = NR * 128
    f32r = mybir.dt.float32r
    f32 = mybir.dt.float32
    f8 = mybir.dt.float8e4
    DR = mybir.MatmulPerfMode.DoubleRow
    add = mybir.AluOpType.add

    nc = bacc.Bacc(trn_type="TRN2")
    with tile.TileContext(nc) as tc:
        with tc.tile_pool(name="dram", bufs=1, space="DRAM") as dram:
            # x/vt in DoubleRow layout [kp*128, 4, cols]: planes 0:2 = hi
            # fp8 split, planes 2:4 = lo split (one DMA per tile)
            xc = dram.tile([D_IN // 2, 4, TC], f8, kind="ExternalInput", name="xc")
            vc = dram.tile([D_IN // 2, 4, RP], f8, kind="ExternalInput", name="vc")
            ut = dram.tile([RP, D_OUT], f32r, kind="ExternalInput", name="ut")
            yT = dram.tile([D_OUT, TC], f32, kind="ExternalOutput", name="yT")

            with (
                tc.tile_pool(name="tsb", bufs=NR) as tpool,
                tc.tile_pool(name="ut0", bufs=1) as u0pool,
            ):
                t_sb = [tpool.tile([128, TC], f32r, name="tsb") for _ in range(NR)]
                # first ut chunk: loads during phase 1 (own address space)
                ut0 = u0pool.tile([128, NR, 512], f32r)

                # single PSUM pool for the whole kernel: 8 x 1-bank tiles
                pspool = tc.alloc_tile_pool(name="ps", bufs=8, space="PSUM")

                # warm-up: no deps (t_sb[0] is only written later); fills the
                # PE pipe + p-state ramp while the first DMAs land
                for _ in range(N_WARM):
                    wp = pspool.tile([128, 512], f32, name="ps")
                    nc.tensor.matmul(
                        wp[:],
                        lhsT=t_sb[0][:, 0:128],
                        rhs=t_sb[0][:, 0:512],
                        start=True,
                        stop=True,
                    )

                # ---- phase 1 (fp8 DoubleRow) ----
                xpool = tc.alloc_tile_pool(name="xk", bufs=2 * KPB)
                vpool = tc.alloc_tile_pool(name="vk", bufs=2 * KPB)

                def new_xv():
                    return (
                        xpool.tile([128, 4, TC], f8, name="xk"),
                        vpool.tile([128, 4, RP], f8, name="vk"),
                    )

                def rows(kp):
                    return slice(kp * 128, (kp + 1) * 128)

                # blocks 0/1 (k-pairs 0..3): DMAs in exact consumption order
                kv01 = [new_xv() for _ in range(2 * KPB)]
                # k-pair 0 leading pieces: first matmul needs x c0 + the lead
                # 128 vt columns
                x0, v0 = kv01[0]
                nc.sync.dma_start(x0[:, :, 0:512], xc[rows(0), :, 0:512])
                nc.sync.dma_start(v0[:, :, 0:128], vc[rows(0), :, 0:128])
                nc.sync.dma_start(v0[:, :, 128:RP], vc[rows(0), :, 128:RP])
                # k-pairs 1..3: vt whole, x first chunk now, rest in quads
                for kp in range(1, 2 * KPB):
                    x_t, v_t = kv01[kp]
                    nc.sync.dma_start(v_t[:], vc[rows(kp), :, :])
                    nc.sync.dma_start(x_t[:, :, 0:512], xc[rows(kp), :, 0:512])
                for n in range(1, NN):
                    sl = slice(n * 512, (n + 1) * 512)
                    for kp in range(2 * KPB):
                        x_t, _ = kv01[kp]
                        nc.sync.dma_start(x_t[:, :, sl], xc[rows(kp), :, sl])
                nc.sync.dma_start(
                    ut0[:], ut[:, 0:512].rearrange("(ko p) f -> p ko f", p=128)
                )

                # uniform n-outer / r-inner blocks, one bank per (n, r) group;
                # per k-pair: 3 hi/lo products accumulated in the same group
                for kb in range(NBP):
                    if kb < 2:
                        kvs = kv01[kb * KPB:(kb + 1) * KPB]
                    else:
                        kvs = []
                        for j in range(KPB):
                            kp = kb * KPB + j
                            x_t, v_t = new_xv()
                            nc.sync.dma_start(x_t[:], xc[rows(kp), :, :])
                            nc.sync.dma_start(v_t[:], vc[rows(kp), :, :])
                            kvs.append((x_t, v_t))
                    for n in range(NN):
                        sl = slice(n * 512, (n + 1) * 512)
                        ps_n = [
                            pspool.tile([128, 512], f32, name="ps") for _ in range(NR)
                        ]
                        HI, LO = slice(0, 2), slice(2, 4)
                        for j in range(KPB):
                            x_t, v_t = kvs[j]
                            for r in range(NR):
                                rsl = slice(r * 128, (r + 1) * 128)
                                for p, (vs, xs) in enumerate(
                                    ((HI, HI), (HI, LO), (LO, HI))
                                ):
                                    nc.tensor.matmul(
                                        ps_n[r][:],
                                        lhsT=v_t[:, vs, rsl],
                                        rhs=x_t[:, xs, sl],
                                        start=(j == 0 and p == 0),
                                        stop=(j == KPB - 1 and p == 2),
                                        perf_mode=DR,
                                    )
                        if kb == 0:
                            for r in range(NR):
                                nc.any.tensor_copy(t_sb[r][:, sl], ps_n[r][:])
                        else:
                            for r in range(NR):
                                nc.any.tensor_tensor(
                                    t_sb[r][:, sl], t_sb[r][:, sl], ps_n[r][:], add
                                )
                vpool.release()
                xpool.release()

                # ---- phase 2 (fp32r: ut stationary, t moving; dout-major) ----
                with (
                    tc.tile_pool(name="utd", bufs=2) as upool,
                    tc.tile_pool(name="ysb", bufs=8) as ypool,
                ):
                    for d in range(ND):
                        if d == 0:
                            ut_t = ut0
                        else:
                            ut_t = upool.tile([128, NR, 512], f32r, name="utd")
                            nc.sync.dma_start(
                                ut_t[:],
                                ut[:, d * 512:(d + 1) * 512].rearrange(
                                    "(ko p) f -> p ko f", p=128
                                ),
                            )
                        for dd in range(4):  # 128-wide dout sub-blocks
                            row = d * 512 + dd * 128
                            for n in range(NN):
                                final = d == ND - 1 and dd == 3 and n == NN - 1
                                # very last chunk: two 256-token sub-groups so
                                # the post-last-matmul drain chain is half as
                                # long (256 keeps fp32r at full rate)
                                subs = ((0, 256), (256, 512)) if final else ((0, 512),)
                                for lo, hi in subs:
                                    pt = pspool.tile([128, 512], f32, name="ps")
                                    for r in range(NR):
                                        nc.tensor.matmul(
                                            pt[:, lo:hi],
                                            lhsT=ut_t[:, r, dd * 128:(dd + 1) * 128],
                                            rhs=t_sb[r][:, n * 512 + lo:n * 512 + hi],
                                            start=(r == 0),
                                            stop=(r == NR - 1),
                                        )
                                    ysb = ypool.tile([128, hi - lo], f32, name="ysb")
                                    nc.any.tensor_copy(ysb[:], pt[:, lo:hi])
                                    nc.sync.dma_start(
                                        yT[row : row + 128,
                                           n * 512 + lo:n * 512 + hi],
                                        ysb[:],
                                    )
                pspool.release()
    nc.compile()
    return nc, (xc.name, vc.name, ut.name), yT.name


def _dr_layout(a2d):
    """[256*KP, C] -> DoubleRow layout [KP*128, 2, C] (k = plane*128 + p)."""
    kp, c = a2d.shape[0] // 256, a2d.shape[1]
    return np.ascontiguousarray(
        a2d.reshape(kp, 2, 128, c).transpose(0, 2, 1, 3).reshape(kp * 128, 2, c)
    )


def _split8cat(a):
    """[kp*128, 2, C] f32 -> [kp*128, 4, C] fp8: planes 0:2 hi, 2:4 lo."""
    import ml_dtypes

    f8 = ml_dtypes.float8_e4m3fn
    hi = a.astype(f8)
    lo = (a - hi.astype(np.float32)).astype(f8)
    return np.ascontiguousarray(np.concatenate([hi, lo], axis=1))


def _prep_maps(input, weight, U, Vh, U_additional, Vh_additional, mask, names, NR):
    xc_n, vc_n, ut_n = names
    RP = NR * 128
    s = weight * weight * mask
    U_eff = np.zeros((D_OUT, RP), np.float32)
    U_eff[:, :R] = U * s[None, :]
    V_eff = np.zeros((RP, D_IN), np.float32)
    V_eff[:R] = Vh
    if NR > R // 128:
        U_eff[:, R : R + A] = U_additional
        V_eff[R : R + A] = Vh_additional
    # vt scaled into fp8 range; inverse folded into ut (exact power of two)
    vc_a = _split8cat(_dr_layout(np.ascontiguousarray(V_eff.T) * np.float32(VSCALE)))
    ut = np.ascontiguousarray(U_eff.T) * np.float32(1.0 / VSCALE)
    x2 = np.asarray(input, dtype=np.float32).reshape(T, D_IN)
    in_maps = []
    for c in range(N_CORES):
        xc_a = _split8cat(_dr_layout(np.ascontiguousarray(x2[c * TC : (c + 1) * TC].T)))
        in_maps.append({xc_n: xc_a, vc_n: vc_a, ut_n: ut})
    return in_maps


def _gather(results, yT_name):
    out = np.empty((T, D_OUT), np.float32)
    for c in range(N_CORES):
        out[c * TC : (c + 1) * TC] = results[c][yT_name].T
    return out.reshape(B, S, D_OUT)


def _pick_nr(U_additional, Vh_additional):
    if not np.asarray(U_additional).any() or not np.asarray(Vh_additional).any():
        return R // 128  # additional term contributes nothing
    return (R + A + 127) // 128


def kernel(input, weight, U, Vh, U_additional, Vh_additional, mask, **_kw):
    from concourse.bass_utils import run_bass_kernel_spmd

    input = np.asarray(input, dtype=np.float32)
    weight = np.asarray(weight, dtype=np.float32)
    U = np.asarray(U, dtype=np.float32)
    Vh = np.asarray(Vh, dtype=np.float32)
    U_additional = np.asarray(U_additional, dtype=np.float32)
    Vh_additional = np.asarray(Vh_additional, dtype=np.float32)
    mask = np.asarray(mask, dtype=np.float32)

    NR = _pick_nr(U_additional, Vh_additional)
    nc, in_names, yT_name = _build(NR)
    in_maps = _prep_maps(
        input, weight, U, Vh, U_additional, Vh_additional, mask, in_names, NR,
    )
    res = run_bass_kernel_spmd(nc, in_maps, core_ids=list(range(N_CORES)))
    return _gather(res.results, yT_name)


# revision 23
# speedup vs baseline: 1.2500x; 1.1113x over previous
"""TRN2 Bass kernel for nn_BSLinear_71159018160311 — fp8 DoubleRow both phases.

out = input @ W.T,  W = U @ diag(weight^2 * mask) @ Vh (+ additional term).

Data-parallel over B*S=16384 tokens across 8 cores (2048 tokens/core), no
collectives. Factorized two-phase form per core:

  phase 1: t = V_eff @ x_c.T  in fp8e4m3 DoubleRow (256-deep contraction,
           0.5 PE cycles/row) with hi/lo splitting of BOTH operands:
           A@B ~= A1B1 + A1B2 + A2B1 (lo*lo dropped), all three products
           accumulated in the same PSUM group -> 0.75 cycles/row at ~8
           significand bits per operand, vs 1.0 for bf16/fp32r.
           vt is pre-scaled by 64 on host (fp8 dynamic range), the inverse
           is folded into ut. PSUM/t_sb stay fp32.
  phase 2: yT_c = U_eff @ t  also in fp8 DoubleRow with hi/lo splitting;
           the last phase-1 block's drains emit t as an fp8 hi/lo pair
           (t carries vt's 64x pre-scale; the host divides y by 64).

Schedule: ONE 8-bank PSUM pool; every accumulation group is [128, 512]
(one bank, one 512-token chunk), drained right after it completes
(bank reuse distance >= 8 groups -> no matmul ever WAR-waits on a drain;
tile-granular WAR deps). Warm-up matmuls (no deps) hold the PE p-state
ramp while the first DMAs land; first-block DMAs are issued in exact
consumption order with the leading pieces split small. The very last
output chunk is drained as two 256-token sub-groups to halve the tail
chain.
"""

import functools

import numpy as np

B, S, D_IN, D_OUT, R, A = 4, 4096, 4096, 4096, 1024, 64
N_CORES = 8
T = B * S
TC = T // N_CORES  # 2048
KP = D_IN // 256  # 16 k-pairs (256-deep DR contraction each)
KPB = 2  # k-pairs per phase-1 block
NBP = KP // KPB  # 8 blocks
NN = TC // 512  # 4
ND = D_OUT // 512  # 8
N_WARM = 8  # warm-up matmuls covering the DMA cold start
VSCALE = 16.0  # vt pre-scale: keeps 16*t under the DVE fp8 clamp (+-256)
USCALE = 32.0  # host pre-scale of ut into fp8 range; y /= VSCALE*USCALE


@functools.lru_cache(maxsize=2)
def _build(NR):
    import concourse.bacc as bacc
    import concourse.mybir as mybir
    import concourse.tile as tile

    RP = NR * 128
    f32r = mybir.dt.float32r
    f32 = mybir.dt.float32
    f8 = mybir.dt.float8e4
    DR = mybir.MatmulPerfMode.DoubleRow
    add = mybir.AluOpType.add
    sub = mybir.AluOpType.subtract

    nc = bacc.Bacc(trn_type="TRN2")
    with tile.TileContext(nc) as tc:
        with tc.tile_pool(name="dram", bufs=1, space="DRAM") as dram:
            # x/vt in DoubleRow layout [kp*128, 4, cols]: planes 0:2 = hi
            # fp8 split, planes 2:4 = lo split (one DMA per tile)
            xc = dram.tile([D_IN // 2, 4, TC], f8, kind="ExternalInput", name="xc")
            vc = dram.tile([D_IN // 2, 4, RP], f8, kind="ExternalInput", name="vc")
            # [p, rp, hi/lo-plane, dout]: p outermost so per-d tiles load as
            # one 3-dim DMA (rp and plane dims merge)
            uc = dram.tile([128, RP // 256, 4, D_OUT], f8, kind="ExternalInput", name="uc")
            yT = dram.tile([D_OUT, TC], f32, kind="ExternalOutput", name="yT")

            with (
                tc.tile_pool(name="tsb", bufs=NR) as tpool,
                tc.tile_pool(name="ut0", bufs=1) as u0pool,
            ):
                t_sb = [tpool.tile([128, TC], f32r, name="tsb") for _ in range(NR)]
                # fp8 DR pair of the final t (planes 0:2 hi, 2:4 lo), written
                # by the last phase-1 block's drains
                t8pool = tc.alloc_tile_pool(name="t8", bufs=NR // 2)
                t8 = [
                    t8pool.tile([128, 4, TC], f8, name="t8")
                    for _ in range(NR // 2)
                ]
                # f32 scratch for the exact fp8->f32 upcast of t's hi split
                cvtpool = tc.alloc_tile_pool(name="cvt", bufs=8)
                # first ut chunk: loads during phase 1 (own address space)
                ut0 = u0pool.tile([128, NR // 2, 4, 512], f8)

                # single PSUM pool for the whole kernel: 8 x 1-bank tiles
                pspool = tc.alloc_tile_pool(name="ps", bufs=8, space="PSUM")

                # warm-up: no deps (t_sb[0] is only written later); fills the
                # PE pipe + p-state ramp while the first DMAs land
                for _ in range(N_WARM):
                    wp = pspool.tile([128, 512], f32, name="ps")
                    nc.tensor.matmul(
                        wp[:],
                        lhsT=t_sb[0][:, 0:128],
                        rhs=t_sb[0][:, 0:512],
                        start=True,
                        stop=True,
                    )

                # ---- phase 1 (fp8 DoubleRow) ----
                xpool = tc.alloc_tile_pool(name="xk", bufs=2 * KPB)
                vpool = tc.alloc_tile_pool(name="vk", bufs=2 * KPB)

                def new_xv():
                    return (
                        xpool.tile([128, 4, TC], f8, name="xk"),
                        vpool.tile([128, 4, RP], f8, name="vk"),
                    )

                def rows(kp):
                    return slice(kp * 128, (kp + 1) * 128)

                # blocks 0/1 (k-pairs 0..3): DMAs in exact consumption order
                kv01 = [new_xv() for _ in range(2 * KPB)]
                # k-pair 0 leading pieces: first matmul needs x c0 + the lead
                # 128 vt columns
                x0, v0 = kv01[0]
                nc.sync.dma_start(x0[:, :, 0:512], xc[rows(0), :, 0:512])
                nc.sync.dma_start(v0[:, :, 0:128], vc[rows(0), :, 0:128])
                nc.sync.dma_start(v0[:, :, 128:RP], vc[rows(0), :, 128:RP])
                # k-pairs 1..3: vt whole, x first chunk now, rest in quads
                for kp in range(1, 2 * KPB):
                    x_t, v_t = kv01[kp]
                    nc.sync.dma_start(v_t[:], vc[rows(kp), :, :])
                    nc.sync.dma_start(x_t[:, :, 0:512], xc[rows(kp), :, 0:512])
                for n in range(1, NN):
                    sl = slice(n * 512, (n + 1) * 512)
                    for kp in range(2 * KPB):
                        x_t, _ = kv01[kp]
                        nc.sync.dma_start(x_t[:, :, sl], xc[rows(kp), :, sl])
                nc.sync.dma_start(ut0[:], uc[:, :, :, 0:512])

                # uniform n-outer / r-inner blocks, one bank per (n, r) group;
                # per k-pair: 3 hi/lo products accumulated in the same group
                for kb in range(NBP):
                    if kb < 2:
                        kvs = kv01[kb * KPB:(kb + 1) * KPB]
                    else:
                        kvs = []
                        for j in range(KPB):
                            kp = kb * KPB + j
                            x_t, v_t = new_xv()
                            nc.sync.dma_start(x_t[:], xc[rows(kp), :, :])
                            nc.sync.dma_start(v_t[:], vc[rows(kp), :, :])
                            kvs.append((x_t, v_t))
                    for n in range(NN):
                        sl = slice(n * 512, (n + 1) * 512)
                        ps_n = [
                            pspool.tile([128, 512], f32, name="ps") for _ in range(NR)
                        ]
                        HI, LO = slice(0, 2), slice(2, 4)
                        for j in range(KPB):
                            x_t, v_t = kvs[j]
                            for r in range(NR):
                                rsl = slice(r * 128, (r + 1) * 128)
                                for p, (vs, xs) in enumerate(
                                    ((HI, HI), (HI, LO), (LO, HI))
                                ):
                                    nc.tensor.matmul(
                                        ps_n[r][:],
                                        lhsT=v_t[:, vs, rsl],
                                        rhs=x_t[:, xs, sl],
                                        start=(j == 0 and p == 0),
                                        stop=(j == KPB - 1 and p == 2),
                                        perf_mode=DR,
                                    )
                        if kb == 0:
                            for r in range(NR):
                                nc.any.tensor_copy(t_sb[r][:, sl], ps_n[r][:])
                        else:
                            for r in range(NR):
                                nc.any.tensor_tensor(
                                    t_sb[r][:, sl], t_sb[r][:, sl], ps_n[r][:], add
                                )
                            if kb == NBP - 1:
                                # final t for this chunk: emit the fp8 hi/lo
                                # pair phase 2 consumes. hi = fp8(t); the lo
                                # subtract goes through an exact fp8->f32
                                # upcast (same-dtype subtract - the direct
                                # f32-fp8 mix mis-rounds on the device)
                                for r in range(NR):
                                    hi8 = t8[r // 2][:, r % 2, sl]
                                    lo8 = t8[r // 2][:, 2 + r % 2, sl]
                                    nc.any.tensor_copy(hi8, t_sb[r][:, sl])
                                    hif = cvtpool.tile([128, 512], f32, name="cvt")
                                    nc.any.tensor_copy(hif[:], hi8)
                                    nc.any.tensor_tensor(
                                        lo8, t_sb[r][:, sl], hif[:], sub
                                    )
                vpool.release()
                xpool.release()

                # ---- phase 2 (fp32r: ut stationary, t moving; dout-major) ----
                with (
                    tc.tile_pool(name="utd", bufs=2) as upool,
                    tc.tile_pool(name="ysb", bufs=8) as ypool,
                ):
                    for d in range(ND):
                        if d == 0:
                            ut_t = ut0
                        else:
                            ut_t = upool.tile([128, NR // 2, 4, 512], f8, name="utd")
                            nc.sync.dma_start(
                                ut_t[:], uc[:, :, :, d * 512:(d + 1) * 512]
                            )
                        for dd in range(4):  # 128-wide dout sub-blocks
                            row = d * 512 + dd * 128
                            for n in range(NN):
                                final = d == ND - 1 and dd == 3 and n == NN - 1
                                # very last chunk: two 256-token sub-groups so
                                # the post-last-matmul drain chain is half as
                                # long (256 keeps fp32r at full rate)
                                subs = ((0, 256), (256, 512)) if final else ((0, 512),)
                                HI, LO = slice(0, 2), slice(2, 4)
                                for lo, hi in subs:
                                    pt = pspool.tile([128, 512], f32, name="ps")
                                    for rp in range(NR // 2):
                                        for p, (us, ts) in enumerate(
                                            ((HI, HI), (HI, LO), (LO, HI))
                                        ):
                                            nc.tensor.matmul(
                                                pt[:, lo:hi],
                                                lhsT=ut_t[:, rp, us, dd * 128:(dd + 1) * 128],
                                                rhs=t8[rp][:, ts, n * 512 + lo:n * 512 + hi],
                                                start=(rp == 0 and p == 0),
                                                stop=(rp == NR // 2 - 1 and p == 2),
                                                perf_mode=DR,
                                            )
                                    ysb = ypool.tile([128, hi - lo], f32, name="ysb")
                                    nc.any.tensor_copy(ysb[:], pt[:, lo:hi])
                                    nc.sync.dma_start(
                                        yT[row : row + 128,
                                           n * 512 + lo:n * 512 + hi],
                                        ysb[:],
                                    )
                pspool.release()
                cvtpool.release()
                t8pool.release()
    nc.compile()
    return nc, (xc.name, vc.name, uc.name), yT.name


def _dr_layout(a2d):
    """[256*KP, C] -> DoubleRow layout [KP*128, 2, C] (k = plane*128 + p)."""
    kp, c = a2d.shape[0] // 256, a2d.shape[1]
    return np.ascontiguousarray(
        a2d.reshape(kp, 2, 128, c).transpose(0, 2, 1, 3).reshape(kp * 128, 2, c)
    )


def _split8cat(a):
    """[kp*128, 2, C] f32 -> [kp*128, 4, C] fp8: planes 0:2 hi, 2:4 lo."""
    import ml_dtypes

    f8 = ml_dtypes.float8_e4m3fn
    hi = a.astype(f8)
    lo = (a - hi.astype(np.float32)).astype(f8)
    return np.ascontiguousarray(np.concatenate([hi, lo], axis=1))


def _prep_maps(input, weight, U, Vh, U_additional, Vh_additional, mask, names, NR):
    xc_n, vc_n, uc_n = names
    RP = NR * 128
    s = weight * weight * mask
    # balanced sqrt(s) split: both fp8 factors stay in e4m3's normal range
    # (s folded one-sidedly would push the small-s columns subnormal)
    rs = np.sqrt(s).astype(np.float32)
    U_eff = np.zeros((D_OUT, RP), np.float32)
    U_eff[:, :R] = U * rs[None, :]
    V_eff = np.zeros((RP, D_IN), np.float32)
    V_eff[:R] = Vh * rs[:, None]
    if NR > R // 128:
        U_eff[:, R : R + A] = U_additional
        V_eff[R : R + A] = Vh_additional
    # vt scaled into fp8 range; inverse folded into ut (exact power of two)
    vc_a = _split8cat(_dr_layout(np.ascontiguousarray(V_eff.T) * np.float32(VSCALE)))
    # t carries vt's 64x pre-scale, ut carries 32x; host divides y by 2048
    uc_a = _split8cat(_dr_layout(np.ascontiguousarray(U_eff.T) * np.float32(USCALE)))
    # [rp*128, 4, D_OUT] -> [128, rp, 4, D_OUT] (partition outermost)
    rp_n = RP // 256
    uc_a = np.ascontiguousarray(
        uc_a.reshape(rp_n, 128, 4, D_OUT).transpose(1, 0, 2, 3)
    )
    x2 = np.asarray(input, dtype=np.float32).reshape(T, D_IN)
    in_maps = []
    for c in range(N_CORES):
        xc_a = _split8cat(_dr_layout(np.ascontiguousarray(x2[c * TC : (c + 1) * TC].T)))
        in_maps.append({xc_n: xc_a, vc_n: vc_a, uc_n: uc_a})
    return in_maps


def _gather(results, yT_name):
    out = np.empty((T, D_OUT), np.float32)
    inv = np.float32(1.0 / (VSCALE * USCALE))
    for c in range(N_CORES):
        out[c * TC : (c + 1) * TC] = results[c][yT_name].T
    out *= inv
    return out.reshape(B, S, D_OUT)


def _pick_nr(U_additional, Vh_additional):
    if not np.asarray(U_additional).any() or not np.asarray(Vh_additional).any():
        return R // 128  # additional term contributes nothing
    return (R + A + 127) // 128


def kernel(input, weight, U, Vh, U_additional, Vh_additional, mask, **_kw):
    from concourse.bass_utils import run_bass_kernel_spmd

    input = np.asarray(input, dtype=np.float32)
    weight = np.asarray(weight, dtype=np.float32)
    U = np.asarray(U, dtype=np.float32)
    Vh = np.asarray(Vh, dtype=np.float32)
    U_additional = np.asarray(U_additional, dtype=np.float32)
    Vh_additional = np.asarray(Vh_additional, dtype=np.float32)
    mask = np.asarray(mask, dtype=np.float32)

    NR = _pick_nr(U_additional, Vh_additional)
    nc, in_names, yT_name = _build(NR)
    in_maps = _prep_maps(
        input, weight, U, Vh, U_additional, Vh_additional, mask, in_names, NR,
    )
    res = run_bass_kernel_spmd(nc, in_maps, core_ids=list(range(N_CORES)))
    return _gather(res.results, yT_name)


# revision 25
# speedup vs baseline: 1.3132x; 1.0506x over previous
"""TRN2 Bass kernel for nn_BSLinear_71159018160311 — fp8 DoubleRow both phases.

out = input @ W.T,  W = U @ diag(weight^2 * mask) @ Vh (+ additional term).

Data-parallel over B*S=16384 tokens across 8 cores (2048 tokens/core), no
collectives. Factorized two-phase form per core:

  phase 1: t = V_eff @ x_c.T  in fp8e4m3 DoubleRow (256-deep contraction,
           0.5 PE cycles/row) with hi/lo splitting of BOTH operands:
           A@B ~= A1B1 + A1B2 + A2B1 (lo*lo dropped), all three products
           accumulated in the same PSUM group -> 0.75 cycles/row at ~8
           significand bits per operand, vs 1.0 for bf16/fp32r.
           vt is pre-scaled by 64 on host (fp8 dynamic range), the inverse
           is folded into ut. PSUM/t_sb stay fp32.
  phase 2: yT_c = U_eff @ t  also in fp8 DoubleRow with hi/lo splitting;
           the last phase-1 block's drains emit t as an fp8 hi/lo pair
           (t carries vt's 64x pre-scale; the host divides y by 64).

Schedule: ONE 8-bank PSUM pool; every accumulation group is [128, 512]
(one bank, one 512-token chunk), drained right after it completes
(bank reuse distance >= 8 groups -> no matmul ever WAR-waits on a drain;
tile-granular WAR deps). Warm-up matmuls (no deps) hold the PE p-state
ramp while the first DMAs land; first-block DMAs are issued in exact
consumption order with the leading pieces split small. The very last
output chunk is drained as two 256-token sub-groups to halve the tail
chain.
"""

import functools

import numpy as np

B, S, D_IN, D_OUT, R, A = 4, 4096, 4096, 4096, 1024, 64
N_CORES = 8
T = B * S
TC = T // N_CORES  # 2048
KP = D_IN // 256  # 16 k-pairs (256-deep DR contraction each)
KPB = 4  # k-pairs per phase-1 block (wide blocks halve accumulator drains)
NBP = KP // KPB  # 4 blocks
NN = TC // 512  # 4
ND = D_OUT // 512  # 8
N_WARM = 8  # warm-up matmuls covering the DMA cold start
VSCALE = 16.0  # vt pre-scale: keeps 16*t under the DVE fp8 clamp (+-256)
USCALE = 32.0  # host pre-scale of ut into fp8 range; y /= VSCALE*USCALE


@functools.lru_cache(maxsize=2)
def _build(NR):
    import concourse.bacc as bacc
    import concourse.mybir as mybir
    import concourse.tile as tile

    RP = NR * 128
    f32r = mybir.dt.float32r
    f32 = mybir.dt.float32
    f8 = mybir.dt.float8e4
    DR = mybir.MatmulPerfMode.DoubleRow
    add = mybir.AluOpType.add
    sub = mybir.AluOpType.subtract

    nc = bacc.Bacc(trn_type="TRN2")
    with tile.TileContext(nc) as tc:
        with tc.tile_pool(name="dram", bufs=1, space="DRAM") as dram:
            # x/vt in DoubleRow layout [kp*128, 4, cols]: planes 0:2 = hi
            # fp8 split, planes 2:4 = lo split (one DMA per tile)
            xc = dram.tile([D_IN // 2, 4, TC], f8, kind="ExternalInput", name="xc")
            vc = dram.tile([D_IN // 2, 4, RP], f8, kind="ExternalInput", name="vc")
            # [p, rp, hi/lo-plane, dout]: p outermost so per-d tiles load as
            # one 3-dim DMA (rp and plane dims merge)
            uc = dram.tile([128, RP // 256, 4, D_OUT], f8, kind="ExternalInput", name="uc")
            yT = dram.tile([D_OUT, TC], f32, kind="ExternalOutput", name="yT")

            with (
                tc.tile_pool(name="tsb", bufs=NR) as tpool,
                tc.tile_pool(name="ut0", bufs=1) as u0pool,
            ):
                t_sb = [tpool.tile([128, TC], f32r, name="tsb") for _ in range(NR)]
                # fp8 DR pair of the final t (planes 0:2 hi, 2:4 lo), written
                # by the last phase-1 block's drains
                t8pool = tc.alloc_tile_pool(name="t8", bufs=NR // 2)
                t8 = [
                    t8pool.tile([128, 4, TC], f8, name="t8")
                    for _ in range(NR // 2)
                ]
                # first ut chunk: loads during phase 1 (own address space)
                ut0 = u0pool.tile([128, NR // 2, 4, 512], f8)

                # single PSUM pool for the whole kernel: 8 x 1-bank tiles
                pspool = tc.alloc_tile_pool(name="ps", bufs=8, space="PSUM")

                # warm-up: no deps (t_sb[0] is only written later); fills the
                # PE pipe + p-state ramp while the first DMAs land
                for _ in range(N_WARM):
                    wp = pspool.tile([128, 512], f32, name="ps")
                    nc.tensor.matmul(
                        wp[:],
                        lhsT=t_sb[0][:, 0:128],
                        rhs=t_sb[0][:, 0:512],
                        start=True,
                        stop=True,
                    )

                # ---- phase 1 (fp8 DoubleRow) ----
                # x as per-(kp, token-chunk) tiles (one block's worth in
                # flight), vt whole tiles double-buffered across blocks
                xpool = tc.alloc_tile_pool(name="xk", bufs=KPB * NN)
                vpool = tc.alloc_tile_pool(name="vk", bufs=2 * KPB)

                def rows(kp):
                    return slice(kp * 128, (kp + 1) * 128)

                def block_tiles(kb, boot):
                    """Allocate + DMA one block's v tiles and x chunk tiles
                    in consumption order (v + chunk-0 first, then chunk
                    quads)."""
                    vts = [vpool.tile([128, 4, RP], f8, name="vk") for _ in range(KPB)]
                    xch = [[None] * NN for _ in range(KPB)]
                    for j in range(KPB):
                        kp = kb * KPB + j
                        for n in range(NN):
                            xch[j][n] = xpool.tile([128, 4, 512], f8, name="xk")
                        if boot and j == 0:
                            # leading pieces: first matmul needs x chunk 0 +
                            # the first 128 vt columns only
                            nc.sync.dma_start(
                                xch[0][0][:], xc[rows(kp), :, 0:512]
                            )
                            nc.sync.dma_start(vts[0][:, :, 0:128], vc[rows(kp), :, 0:128])
                            nc.sync.dma_start(vts[0][:, :, 128:RP], vc[rows(kp), :, 128:RP])
                        else:
                            nc.sync.dma_start(vts[j][:], vc[rows(kp), :, :])
                            nc.sync.dma_start(
                                xch[j][0][:], xc[rows(kp), :, 0:512]
                            )
                    for n in range(1, NN):
                        sl = slice(n * 512, (n + 1) * 512)
                        for j in range(KPB):
                            kp = kb * KPB + j
                            nc.sync.dma_start(xch[j][n][:], xc[rows(kp), :, sl])
                    return vts, xch

                # block 0 pre-issued before ut0 so the boot path is clean
                vts0, xch0 = block_tiles(0, boot=True)
                nc.sync.dma_start(ut0[:], uc[:, :, :, 0:512])

                # uniform n-outer / r-inner blocks, one bank per (n, r) group;
                # per k-pair: 3 hi/lo products accumulated in the same group
                for kb in range(NBP):
                    if kb == 0:
                        vts, xch = vts0, xch0
                    else:
                        vts, xch = block_tiles(kb, boot=False)
                    for n in range(NN):
                        sl = slice(n * 512, (n + 1) * 512)
                        ps_n = [
                            pspool.tile([128, 512], f32, name="ps") for _ in range(NR)
                        ]
                        HI, LO = slice(0, 2), slice(2, 4)
                        for j in range(KPB):
                            x_t, v_t = xch[j][n], vts[j]
                            for r in range(NR):
                                rsl = slice(r * 128, (r + 1) * 128)
                                for p, (vs, xs) in enumerate(
                                    ((HI, HI), (HI, LO), (LO, HI))
                                ):
                                    nc.tensor.matmul(
                                        ps_n[r][:],
                                        lhsT=v_t[:, vs, rsl],
                                        rhs=x_t[:, xs, :],
                                        start=(j == 0 and p == 0),
                                        stop=(j == KPB - 1 and p == 2),
                                        perf_mode=DR,
                                    )
                        if kb == 0:
                            for r in range(NR):
                                nc.any.tensor_copy(t_sb[r][:, sl], ps_n[r][:])
                        else:
                            for r in range(NR):
                                nc.any.tensor_tensor(
                                    t_sb[r][:, sl], t_sb[r][:, sl], ps_n[r][:], add
                                )
                            if kb == NBP - 1:
                                # final t for this chunk: emit the fp8 hi/lo
                                # pair phase 2 consumes. hi = fp8(t); the lo
                                # subtract goes through an exact fp8->f32
                                # upcast (same-dtype subtract - the direct
                                # f32-fp8 mix mis-rounds on the device)
                                for r in range(NR):
                                    hi8 = t8[r // 2][:, r % 2, sl]
                                    lo8 = t8[r // 2][:, 2 + r % 2, sl]
                                    nc.any.tensor_copy(hi8, t_sb[r][:, sl])
                                    nc.any.tensor_tensor(
                                        lo8, t_sb[r][:, sl], hi8, sub
                                    )
                vpool.release()
                xpool.release()

                # ---- phase 2 (fp32r: ut stationary, t moving; dout-major) ----
                with (
                    tc.tile_pool(name="utd", bufs=2) as upool,
                    tc.tile_pool(name="ysb", bufs=6) as ypool,
                ):
                    for d in range(ND):
                        if d == 0:
                            ut_t = ut0
                        else:
                            ut_t = upool.tile([128, NR // 2, 4, 512], f8, name="utd")
                            nc.sync.dma_start(
                                ut_t[:], uc[:, :, :, d * 512:(d + 1) * 512]
                            )
                        for dd in range(4):  # 128-wide dout sub-blocks
                            row = d * 512 + dd * 128
                            for n in range(NN):
                                final = d == ND - 1 and dd == 3 and n == NN - 1
                                # very last chunk: two 256-token sub-groups so
                                # the post-last-matmul drain chain is half as
                                # long (256 keeps fp32r at full rate)
                                subs = ((0, 256), (256, 512)) if final else ((0, 512),)
                                HI, LO = slice(0, 2), slice(2, 4)
                                for lo, hi in subs:
                                    pt = pspool.tile([128, 512], f32, name="ps")
                                    for rp in range(NR // 2):
                                        for p, (us, ts) in enumerate(
                                            ((HI, HI), (HI, LO), (LO, HI))
                                        ):
                                            nc.tensor.matmul(
                                                pt[:, lo:hi],
                                                lhsT=ut_t[:, rp, us, dd * 128:(dd + 1) * 128],
                                                rhs=t8[rp][:, ts, n * 512 + lo:n * 512 + hi],
                                                start=(rp == 0 and p == 0),
                                                stop=(rp == NR // 2 - 1 and p == 2),
                                                perf_mode=DR,
                                            )
                                    ysb = ypool.tile([128, hi - lo], f32, name="ysb")
                                    nc.any.tensor_copy(ysb[:], pt[:, lo:hi])
                                    nc.sync.dma_start(
                                        yT[row : row + 128,
                                           n * 512 + lo:n * 512 + hi],
                                        ysb[:],
                                    )
                pspool.release()
                t8pool.release()
    nc.compile()
    return nc, (xc.name, vc.name, uc.name), yT.name


def _dr_layout(a2d):
    """[256*KP, C] -> DoubleRow layout [KP*128, 2, C] (k = plane*128 + p)."""
    kp, c = a2d.shape[0] // 256, a2d.shape[1]
    return np.ascontiguousarray(
        a2d.reshape(kp, 2, 128, c).transpose(0, 2, 1, 3).reshape(kp * 128, 2, c)
    )


def _split8cat(a):
    """[kp*128, 2, C] f32 -> [kp*128, 4, C] fp8: planes 0:2 hi, 2:4 lo."""
    import ml_dtypes

    f8 = ml_dtypes.float8_e4m3fn
    hi = a.astype(f8)
    lo = (a - hi.astype(np.float32)).astype(f8)
    return np.ascontiguousarray(np.concatenate([hi, lo], axis=1))


def _prep_maps(input, weight, U, Vh, U_additional, Vh_additional, mask, names, NR):
    xc_n, vc_n, uc_n = names
    RP = NR * 128
    s = weight * weight * mask
    # balanced sqrt(s) split: both fp8 factors stay in e4m3's normal range
    # (s folded one-sidedly would push the small-s columns subnormal)
    rs = np.sqrt(s).astype(np.float32)
    U_eff = np.zeros((D_OUT, RP), np.float32)
    U_eff[:, :R] = U * rs[None, :]
    V_eff = np.zeros((RP, D_IN), np.float32)
    V_eff[:R] = Vh * rs[:, None]
    if NR > R // 128:
        U_eff[:, R : R + A] = U_additional
        V_eff[R : R + A] = Vh_additional
    # vt scaled into fp8 range; inverse folded into ut (exact power of two)
    vc_a = _split8cat(_dr_layout(np.ascontiguousarray(V_eff.T) * np.float32(VSCALE)))
    # t carries vt's 64x pre-scale, ut carries 32x; host divides y by 2048
    uc_a = _split8cat(_dr_layout(np.ascontiguousarray(U_eff.T) * np.float32(USCALE)))
    # [rp*128, 4, D_OUT] -> [128, rp, 4, D_OUT] (partition outermost)
    rp_n = RP // 256
    uc_a = np.ascontiguousarray(
        uc_a.reshape(rp_n, 128, 4, D_OUT).transpose(1, 0, 2, 3)
    )
    x2 = np.asarray(input, dtype=np.float32).reshape(T, D_IN)
    in_maps = []
    for c in range(N_CORES):
        xc_a = _split8cat(_dr_layout(np.ascontiguousarray(x2[c * TC : (c + 1) * TC].T)))
        in_maps.append({xc_n: xc_a, vc_n: vc_a, uc_n: uc_a})
    return in_maps


def _gather(results, yT_name):
    out = np.empty((T, D_OUT), np.float32)
    inv = np.float32(1.0 / (VSCALE * USCALE))
    for c in range(N_CORES):
        out[c * TC : (c + 1) * TC] = results[c][yT_name].T
    out *= inv
    return out.reshape(B, S, D_OUT)


def _pick_nr(U_additional, Vh_additional):
    if not np.asarray(U_additional).any() or not np.asarray(Vh_additional).any():
        return R // 128  # additional term contributes nothing
    return (R + A + 127) // 128


def kernel(input, weight, U, Vh, U_additional, Vh_additional, mask, **_kw):
    from concourse.bass_utils import run_bass_kernel_spmd

    input = np.asarray(input, dtype=np.float32)
    weight = np.asarray(weight, dtype=np.float32)
    U = np.asarray(U, dtype=np.float32)
    Vh = np.asarray(Vh, dtype=np.float32)
    U_additional = np.asarray(U_additional, dtype=np.float32)
    Vh_additional = np.asarray(Vh_additional, dtype=np.float32)
    mask = np.asarray(mask, dtype=np.float32)

    NR = _pick_nr(U_additional, Vh_additional)
    nc, in_names, yT_name = _build(NR)
    in_maps = _prep_maps(
        input, weight, U, Vh, U_additional, Vh_additional, mask, in_names, NR,
    )
    res = run_bass_kernel_spmd(nc, in_maps, core_ids=list(range(N_CORES)))
    return _gather(res.results, yT_name)
